# revision 6
# baseline (speedup 1.0000x reference)
"""BankedLinear (MoE-style banked linear) Trainium2 kernel.

Reference computation (per token t, with k=2 selected banks):
    out[t] = sum_k prob[t,k] * (x[t] @ W[sel[t,k]] + bias[sel[t,k]])

Strategy (expert-parallel over 8 NeuronCores):
  - Core c owns banks [8c, 8c+8).  Its weight slab dominates HBM traffic;
    each bank is read exactly once system-wide (4 MB/core as bf16).
  - Host routes token-bank pairs to cores by selected bank, pre-scales each
    gathered token row by its probability, transposes to [in_feature, slot],
    and pads to CAP=32 slots per bank.
  - Precision: the harness gate is rel_err < 2e-2, so x and W are cast to
    plain bf16 (measured end-to-end error ~3e-3).  This halves the weight
    stream vs an fp32-accurate hi/lo split and needs one matmul per
    (bank, k-chunk) instead of three.
  - Weights are host-swizzled to a single [128, BPC*KC*OUT] slab whose free
    index is (bank, kc, out): any span of banks is one contiguous 2D DMA.
  - DMA plan: the 8 per-bank weight DMAs (512 KB each) stream FIFO on the
    sync (SP HWDGE) ring at ~350 GB/s; the x dispatch and the per-quad
    output stores ride the scalar (ACT HWDGE) ring so they never stall the
    weight stream.
  - PSUM: four banks share one [128, OUT] PSUM tile at partition offsets
    0/32/64/96 (tile_position column groups), so one full-width DVE cast
    and one 128 KB store cover four banks.
  - PE warm-up: the PE clock sits at 1.2 GHz until the HAM sees ~3.4 us of
    sustained activity.  A burst of dummy matmuls on a zeroed scratch tile
    warms it during the DMA head so real matmuls run at 2.4 GHz and keep
    pace with the weight stream.
  - Bias is folded in on the host (one gather + multiply-add over 1024
    pairs); host scatter-adds the per-pair device results into the output.

Fixed shapes: B=2, T=256, K=2, IN=OUT=512, NB=64 banks, 8 cores.
Capacity: 32 slots/bank (binomial mean 16, sd ~4; overflow pairs — none for
realistic routing — are handled exactly on the host as a fallback).
"""

import numpy as np
from contextlib import ExitStack

B, T, KSEL = 2, 256, 2
IN, OUT, NB = 512, 512, 64
NCORES = 8
BPC = NB // NCORES          # banks per core = 8
CAP = 32                    # padded token slots per bank
SLOTS = BPC * CAP           # 256 dispatch rows per core
PCHUNK = 128                # contraction chunk (SBUF partition dim)
KC = IN // PCHUNK           # 4 contraction chunks
QUADS = BPC // 4            # 4 banks share one PSUM tile / output store
NWARM = 10                  # dummy matmuls to lift the HAM clock gate

_cache = {}


def _build_nc():
    """Build the Bass/Tile program (one SPMD NeuronCore program)."""
    import concourse.tile as tile
    import concourse.mybir as mybir
    from concourse import bacc

    f32 = mybir.dt.float32
    bf16 = mybir.dt.bfloat16
    nc = bacc.Bacc("TRN2", target_bir_lowering=False, debug=False,
                   num_devices=NCORES)
    # host-pre-swizzled SBUF layouts: partition dim first, contiguous free dim
    xt = nc.dram_tensor("xt", [PCHUNK, KC * SLOTS], bf16,
                        kind="ExternalInput").ap()
    w = nc.dram_tensor("w", [PCHUNK, BPC * KC * OUT], bf16,
                       kind="ExternalInput").ap()
    y = nc.dram_tensor("y", [SLOTS, OUT], bf16, kind="ExternalOutput").ap()

    from concourse.tile import add_dep_helper

    def chain(dep_chain, binst, reason):
        # pin scheduler order: binst depends on the previous link
        if dep_chain:
            add_dep_helper(binst.ins, dep_chain[-1].ins, sync=False,
                           reason=reason)
        dep_chain.append(binst)

    with tile.TileContext(nc) as tc:
        with ExitStack() as ctx:
            xpool = ctx.enter_context(tc.tile_pool(name="xp", bufs=1))
            wpool = ctx.enter_context(tc.tile_pool(name="wp", bufs=BPC))
            ypool = ctx.enter_context(tc.tile_pool(name="yp", bufs=QUADS))
            pspool = ctx.enter_context(
                tc.tile_pool(name="ps", bufs=QUADS + 1, space="PSUM"))

            # x dispatch rides the scalar (ACT) ring, concurrent with the
            # weight stream on the sync (SP) ring
            x_sb = xpool.tile([PCHUNK, KC * SLOTS], bf16, tag="x")
            sq = []    # scalar-ring chain: x, then per-quad y stores
            chain(sq, nc.scalar.dma_start(x_sb[:], xt[:]), "x first on ACT")

            # Weight stream: one 512 KB DMA per bank, except the LAST bank
            # which arrives as four 128 KB kc-chunks so its matmuls can
            # chase the stream and the tail only waits on the final chunk.
            wq = []    # sync-ring DMA chain (keeps FIFO = compute order)
            wts = []   # per bank: list of (tile, col_offset) per kc chunk
            for j in range(BPC):
                if j < BPC - 1:
                    w_t = wpool.tile([PCHUNK, KC * OUT], bf16, tag="w",
                                     name=f"w{j}")
                    chain(wq, nc.sync.dma_start(
                        w_t[:], w[:, j * KC * OUT:(j + 1) * KC * OUT]),
                        "weight ring order")
                    wts.append([(w_t, kc * OUT) for kc in range(KC)])
                else:
                    chunks = []
                    for kc in range(KC):
                        w_t = wpool.tile([PCHUNK, OUT], bf16, tag="wk",
                                         name=f"w{j}k{kc}")
                        c0 = (j * KC + kc) * OUT
                        chain(wq, nc.sync.dma_start(
                            w_t[:], w[:, c0:c0 + OUT]), "weight ring order")
                        chunks.append((w_t, 0))
                    wts.append(chunks)

            # one output staging tile per bank (bank 7: two column halves)
            ysbs = [ypool.tile([CAP, OUT], bf16, tag="y", name=f"ysb{j}")
                    for j in range(BPC - 1)]
            y7a = ypool.tile([CAP, OUT // 2], bf16, tag="y7", name="y7a")
            y7b = ypool.tile([CAP, OUT // 2], bf16, tag="y7", name="y7b")

            # PE warm-up: dummy matmuls on a zeroed scratch tile, ordered
            # before the real matmuls.  They run during the DMA head and
            # flip the HAM clock gate from 1.2 to 2.4 GHz.
            warm_x = xpool.tile([PCHUNK, OUT], bf16, tag="wx")
            warm_ps = pspool.tile([CAP, OUT], f32, tag="wps")
            nc.gpsimd.memset(warm_x[:], 0.0)
            mq = []    # PE order chain: warm-up first, then bank order
            for i in range(NWARM):
                mm = nc.tensor.matmul(warm_ps[:], warm_x[:, :CAP],
                                      warm_x[:, :OUT],
                                      start=True, stop=True,
                                      skip_group_check=True)
                chain(mq, mm, "warmup order")

            # Four banks per quad: bank b lands in PE column group b
            # (tile_position) and PSUM partitions 32b..32b+32 of ONE shared
            # [128, OUT] PSUM tile, so a single full-width DVE cast and one
            # 128 KB store cover the whole quad.
            for qd in range(QUADS):
                ps = pspool.tile([128, OUT], f32, tag="ps", name=f"ps{qd}")
                for b in range(4):
                    j = 4 * qd + b
                    for kc in range(KC):
                        xs = slice(kc * SLOTS + j * CAP,
                                   kc * SLOTS + (j + 1) * CAP)
                        w_t, c0 = wts[j][kc]
                        mm = nc.tensor.matmul(
                            ps[b * CAP:(b + 1) * CAP, :],
                            x_sb[:, xs], w_t[:, c0:c0 + OUT],
                            start=(kc == 0), stop=(kc == KC - 1),
                            tile_position=(0, b * CAP),
                            skip_group_check=True)
                        if kc == 0:
                            chain(mq, mm, "bank compute order")
                    # per-bank cast + store as soon as the bank's group
                    # stops; only bank 7's chain sits on the kernel tail
                    psb = ps[b * CAP:(b + 1) * CAP, :]
                    if j < BPC - 1:
                        nc.vector.tensor_copy(ysbs[j][:], psb)
                        chain(sq, nc.scalar.dma_start(
                            y[j * CAP:(j + 1) * CAP, :], ysbs[j][:]),
                            "y store order")
                    else:
                        # column-split: store of the first half overlaps
                        # the cast of the second half
                        h = OUT // 2
                        nc.vector.tensor_copy(
                            y7a[:], ps[b * CAP:(b + 1) * CAP, 0:h])
                        chain(sq, nc.scalar.dma_start(
                            y[j * CAP:(j + 1) * CAP, 0:h], y7a[:]),
                            "y store order")
                        nc.vector.tensor_copy(
                            y7b[:], ps[b * CAP:(b + 1) * CAP, h:OUT])
                        chain(sq, nc.scalar.dma_start(
                            y[j * CAP:(j + 1) * CAP, h:OUT], y7b[:]),
                            "y store order")
    nc.compile()
    return nc


def _get_nc():
    if "nc" not in _cache:
        _cache["nc"] = _build_nc()
    return _cache["nc"]


def _bf16(a32):
    import ml_dtypes
    return a32.astype(ml_dtypes.bfloat16)


def _swizzle_x(xt):
    """[IN, SLOTS] -> [128, KC*SLOTS] with free index (kc, slot)."""
    return np.ascontiguousarray(
        xt.reshape(KC, PCHUNK, SLOTS).transpose(1, 0, 2).reshape(
            PCHUNK, KC * SLOTS))


def _swizzle_w(w):
    """[BPC, IN, OUT] -> [128, BPC*KC*OUT] with free index (bank, kc, out)."""
    return np.ascontiguousarray(
        w.reshape(BPC, KC, PCHUNK, OUT).transpose(2, 0, 1, 3).reshape(
            PCHUNK, BPC * KC * OUT))


def _route(X, sel, prob):
    """Group token-bank pairs by bank, build per-core dispatch arrays.

    Returns (slot_tok [NCORES,SLOTS] int64 (-1=pad), slot_p, overflow list
    of (token, bank, prob))."""
    NT = X.shape[0]
    pair_tok = np.repeat(np.arange(NT, dtype=np.int64), KSEL)
    pair_bank = sel.reshape(-1)
    pair_p = prob.reshape(-1)

    order = np.argsort(pair_bank, kind="stable")
    counts = np.bincount(pair_bank, minlength=NB)
    starts = np.concatenate(([0], np.cumsum(counts)))

    slot_tok = np.full((NCORES, SLOTS), -1, dtype=np.int64)
    slot_p = np.zeros((NCORES, SLOTS), dtype=np.float32)
    overflow = []
    for b in range(NB):
        c, j = divmod(b, BPC)
        s0, s1 = starts[b], starts[b + 1]
        take = min(s1 - s0, CAP)
        idx = order[s0:s0 + take]
        slot_tok[c, j * CAP: j * CAP + take] = pair_tok[idx]
        slot_p[c, j * CAP: j * CAP + take] = pair_p[idx]
        for i in order[s0 + take:s1]:
            overflow.append((int(pair_tok[i]), b, float(pair_p[i])))
    return slot_tok, slot_p, overflow


def _combine(ys, slot_tok, X, sel, prob, weights, bias, overflow):
    NT = X.shape[0]
    out = np.zeros((NT, OUT), dtype=np.float32)
    for c in range(NCORES):
        tok = slot_tok[c]
        valid = tok >= 0
        np.add.at(out, tok[valid], ys[c][valid].astype(np.float32))
    # bias term for every pair (device computes x @ W only)
    for k in range(KSEL):
        out += prob[:, k, None] * bias[sel[:, k]]
    # exact host fallback for capacity-overflow pairs (expected: none)
    for t, b, p in overflow:
        out[t] += p * (X[t] @ weights[b])
    return out


def _run_device(in_maps, trace=False, **kwargs):
    from concourse.bass_utils import run_bass_kernel_spmd
    return run_bass_kernel_spmd(_get_nc(), in_maps,
                                core_ids=list(range(NCORES)),
                                trace=trace, **kwargs)


def kernel(_trace=False, _bass_results=None, **inputs):
    tensor = np.asarray(inputs["tensor"], dtype=np.float32)
    sel = np.asarray(inputs["bank_selections"]).astype(np.int64)
    prob = np.asarray(inputs["bank_probabilities"], dtype=np.float32)
    weights = np.asarray(inputs["weights"], dtype=np.float32)
    bias = np.asarray(inputs["bias"], dtype=np.float32)

    NT = tensor.shape[0] * tensor.shape[1]
    X = tensor.reshape(NT, IN)
    sel2 = sel.reshape(NT, KSEL)
    prob2 = prob.reshape(NT, KSEL)

    slot_tok, slot_p, overflow = _route(X, sel2, prob2)

    in_maps = []
    for c in range(NCORES):
        tok = slot_tok[c]
        rows = X[np.where(tok >= 0, tok, 0)] * slot_p[c][:, None]
        xt = np.ascontiguousarray(rows.T)              # [IN, SLOTS] fp32
        w32 = weights[c * BPC:(c + 1) * BPC]           # (8, 512, 512) fp32
        in_maps.append({
            "xt": _bf16(_swizzle_x(xt)),
            "w": _bf16(_swizzle_w(w32)),
        })

    res = _run_device(in_maps, trace=_trace)
    if _bass_results is not None:
        _bass_results.append(res)
    ys = [res.results[c]["y"] for c in range(NCORES)]

    out = _combine(ys, slot_tok, X, sel2, prob2, weights, bias, overflow)
    return out.reshape(tensor.shape[0], tensor.shape[1], OUT)


# revision 7
# speedup vs baseline: 1.0806x; 1.0806x over previous
"""BankedLinear (MoE-style banked linear) Trainium2 kernel.

Reference computation (per token t, with k=2 selected banks):
    out[t] = sum_k prob[t,k] * (x[t] @ W[sel[t,k]] + bias[sel[t,k]])

Strategy (expert-parallel over 8 NeuronCores):
  - Core c owns banks [8c, 8c+8).  Its weight slab dominates HBM traffic;
    each bank is read exactly once system-wide (4 MB/core as bf16).
  - Host routes token-bank pairs to cores by selected bank, pre-scales each
    gathered token row by its probability, transposes to [in_feature, slot],
    and pads to CAP=32 slots per bank.
  - Precision: the harness gate is rel_err < 2e-2, so x and W are cast to
    plain bf16 (measured end-to-end error ~3e-3).  This halves the weight
    stream vs an fp32-accurate hi/lo split and needs one matmul per
    (bank, k-chunk) instead of three.
  - Weights are host-swizzled to a single [128, BPC*KC*OUT] slab whose free
    index is (bank, kc, out): any span of banks is one contiguous 2D DMA.
  - DMA plan: the 8 per-bank weight DMAs (512 KB each) stream FIFO on the
    sync (SP HWDGE) ring at ~350 GB/s; the x dispatch and the per-quad
    output stores ride the scalar (ACT HWDGE) ring so they never stall the
    weight stream.
  - PSUM: four banks share one [128, OUT] PSUM tile at partition offsets
    0/32/64/96 (tile_position column groups), so one full-width DVE cast
    and one 128 KB store cover four banks.
  - PE warm-up: the PE clock sits at 1.2 GHz until the HAM sees ~3.4 us of
    sustained activity.  A burst of dummy matmuls on a zeroed scratch tile
    warms it during the DMA head so real matmuls run at 2.4 GHz and keep
    pace with the weight stream.
  - Bias is folded in on the host (one gather + multiply-add over 1024
    pairs); host scatter-adds the per-pair device results into the output.

Fixed shapes: B=2, T=256, K=2, IN=OUT=512, NB=64 banks, 8 cores.
Capacity: 32 slots/bank (binomial mean 16, sd ~4; overflow pairs — none for
realistic routing — are handled exactly on the host as a fallback).
"""

import numpy as np
from contextlib import ExitStack

B, T, KSEL = 2, 256, 2
IN, OUT, NB = 512, 512, 64
NCORES = 8
BPC = NB // NCORES          # banks per core = 8
CAP = 32                    # padded token slots per bank
SLOTS = BPC * CAP           # 256 dispatch rows per core
PCHUNK = 128                # contraction chunk (SBUF partition dim)
KC = IN // PCHUNK           # 4 contraction chunks
QUADS = BPC // 4            # 4 banks share one PSUM tile / output store
NWARM = 10                  # dummy matmuls to lift the HAM clock gate

_cache = {}


def _build_nc():
    """Build the Bass/Tile program (one SPMD NeuronCore program)."""
    import concourse.tile as tile
    import concourse.mybir as mybir
    from concourse import bacc

    f32 = mybir.dt.float32
    bf16 = mybir.dt.bfloat16
    nc = bacc.Bacc("TRN2", target_bir_lowering=False, debug=False,
                   num_devices=NCORES)
    # host-pre-swizzled SBUF layouts: partition dim first, contiguous free dim
    xt = nc.dram_tensor("xt", [PCHUNK, KC * SLOTS], bf16,
                        kind="ExternalInput").ap()
    w = nc.dram_tensor("w", [PCHUNK, BPC * KC * OUT], bf16,
                       kind="ExternalInput").ap()
    y = nc.dram_tensor("y", [SLOTS, OUT], bf16, kind="ExternalOutput").ap()

    from concourse.tile import add_dep_helper

    def chain(dep_chain, binst, reason):
        # pin scheduler order: binst depends on the previous link
        if dep_chain:
            add_dep_helper(binst.ins, dep_chain[-1].ins, sync=False,
                           reason=reason)
        dep_chain.append(binst)

    with tile.TileContext(nc) as tc:
        with ExitStack() as ctx:
            xpool = ctx.enter_context(tc.tile_pool(name="xp", bufs=1))
            wpool = ctx.enter_context(tc.tile_pool(name="wp", bufs=BPC))
            ypool = ctx.enter_context(tc.tile_pool(name="yp", bufs=BPC + 2))
            pspool = ctx.enter_context(
                tc.tile_pool(name="ps", bufs=QUADS + 1, space="PSUM"))

            # x dispatch rides the scalar (ACT) ring, concurrent with the
            # weight stream on the sync (SP) ring
            x_sb = xpool.tile([PCHUNK, KC * SLOTS], bf16, tag="x")
            sq = []    # scalar-ring chain: x, then per-quad y stores
            chain(sq, nc.scalar.dma_start(x_sb[:], xt[:]), "x first on ACT")

            # Weight stream: one 512 KB DMA per bank, except the LAST bank
            # which arrives as four 128 KB kc-chunks so its matmuls can
            # chase the stream and the tail only waits on the final chunk.
            wq = []    # sync-ring DMA chain (keeps FIFO = compute order)
            wts = []   # per bank: list of (tile, col_offset) per kc chunk
            for j in range(BPC):
                if j < BPC - 1:
                    w_t = wpool.tile([PCHUNK, KC * OUT], bf16, tag="w",
                                     name=f"w{j}")
                    chain(wq, nc.sync.dma_start(
                        w_t[:], w[:, j * KC * OUT:(j + 1) * KC * OUT]),
                        "weight ring order")
                    wts.append([(w_t, kc * OUT) for kc in range(KC)])
                else:
                    chunks = []
                    for kc in range(KC):
                        w_t = wpool.tile([PCHUNK, OUT], bf16, tag="wk",
                                         name=f"w{j}k{kc}")
                        c0 = (j * KC + kc) * OUT
                        chain(wq, nc.sync.dma_start(
                            w_t[:], w[:, c0:c0 + OUT]), "weight ring order")
                        chunks.append((w_t, 0))
                    wts.append(chunks)

            # one output staging tile per bank (bank 7: two column halves)
            ysbs = [ypool.tile([CAP, OUT], bf16, tag="y", name=f"ysb{j}")
                    for j in range(BPC - 1)]
            y7a = ypool.tile([CAP, OUT // 2], bf16, tag="y7", name="y7a")
            y7b = ypool.tile([CAP, OUT // 2], bf16, tag="y7", name="y7b")

            # PE warm-up: dummy matmuls on a zeroed scratch tile, ordered
            # before the real matmuls.  They run during the DMA head and
            # flip the HAM clock gate from 1.2 to 2.4 GHz.
            warm_x = xpool.tile([PCHUNK, OUT], bf16, tag="wx")
            warm_ps = pspool.tile([CAP, OUT], f32, tag="wps")
            nc.gpsimd.memset(warm_x[:], 0.0)
            mq = []    # PE order chain: warm-up first, then bank order
            for i in range(NWARM):
                mm = nc.tensor.matmul(warm_ps[:], warm_x[:, :CAP],
                                      warm_x[:, :OUT],
                                      start=True, stop=True,
                                      skip_group_check=True)
                chain(mq, mm, "warmup order")

            # Four banks per quad: bank b lands in PE column group b
            # (tile_position) and PSUM partitions 32b..32b+32 of ONE shared
            # [128, OUT] PSUM tile, so a single full-width DVE cast and one
            # 128 KB store cover the whole quad.
            for qd in range(QUADS):
                ps = pspool.tile([128, OUT], f32, tag="ps", name=f"ps{qd}")
                for b in range(4):
                    j = 4 * qd + b
                    for kc in range(KC):
                        xs = slice(kc * SLOTS + j * CAP,
                                   kc * SLOTS + (j + 1) * CAP)
                        w_t, c0 = wts[j][kc]
                        mm = nc.tensor.matmul(
                            ps[b * CAP:(b + 1) * CAP, :],
                            x_sb[:, xs], w_t[:, c0:c0 + OUT],
                            start=(kc == 0), stop=(kc == KC - 1),
                            tile_position=(0, b * CAP),
                            skip_group_check=True)
                        if kc == 0:
                            chain(mq, mm, "bank compute order")
                    # per-bank cast + store as soon as the bank's group
                    # stops; only bank 7's chain sits on the kernel tail
                    psb = ps[b * CAP:(b + 1) * CAP, :]
                    if j < BPC - 1:
                        nc.vector.tensor_copy(ysbs[j][:], psb)
                        chain(sq, nc.scalar.dma_start(
                            y[j * CAP:(j + 1) * CAP, :], ysbs[j][:]),
                            "y store order")
                    else:
                        # column-split: store of the first half overlaps
                        # the cast of the second half
                        h = OUT // 2
                        nc.vector.tensor_copy(
                            y7a[:], ps[b * CAP:(b + 1) * CAP, 0:h])
                        chain(sq, nc.scalar.dma_start(
                            y[j * CAP:(j + 1) * CAP, 0:h], y7a[:]),
                            "y store order")
                        nc.vector.tensor_copy(
                            y7b[:], ps[b * CAP:(b + 1) * CAP, h:OUT])
                        chain(sq, nc.scalar.dma_start(
                            y[j * CAP:(j + 1) * CAP, h:OUT], y7b[:]),
                            "y store order")
    nc.compile()
    return nc


def _get_nc():
    if "nc" not in _cache:
        _cache["nc"] = _build_nc()
    return _cache["nc"]


def _bf16(a32):
    import ml_dtypes
    return a32.astype(ml_dtypes.bfloat16)


def _swizzle_x(xt):
    """[IN, SLOTS] -> [128, KC*SLOTS] with free index (kc, slot)."""
    return np.ascontiguousarray(
        xt.reshape(KC, PCHUNK, SLOTS).transpose(1, 0, 2).reshape(
            PCHUNK, KC * SLOTS))


def _swizzle_w(w):
    """[BPC, IN, OUT] -> [128, BPC*KC*OUT] with free index (bank, kc, out)."""
    return np.ascontiguousarray(
        w.reshape(BPC, KC, PCHUNK, OUT).transpose(2, 0, 1, 3).reshape(
            PCHUNK, BPC * KC * OUT))


def _route(X, sel, prob):
    """Group token-bank pairs by bank, build per-core dispatch arrays.

    Returns (slot_tok [NCORES,SLOTS] int64 (-1=pad), slot_p, overflow list
    of (token, bank, prob))."""
    NT = X.shape[0]
    pair_tok = np.repeat(np.arange(NT, dtype=np.int64), KSEL)
    pair_bank = sel.reshape(-1)
    pair_p = prob.reshape(-1)

    order = np.argsort(pair_bank, kind="stable")
    counts = np.bincount(pair_bank, minlength=NB)
    starts = np.concatenate(([0], np.cumsum(counts)))

    slot_tok = np.full((NCORES, SLOTS), -1, dtype=np.int64)
    slot_p = np.zeros((NCORES, SLOTS), dtype=np.float32)
    overflow = []
    for b in range(NB):
        c, j = divmod(b, BPC)
        s0, s1 = starts[b], starts[b + 1]
        take = min(s1 - s0, CAP)
        idx = order[s0:s0 + take]
        slot_tok[c, j * CAP: j * CAP + take] = pair_tok[idx]
        slot_p[c, j * CAP: j * CAP + take] = pair_p[idx]
        for i in order[s0 + take:s1]:
            overflow.append((int(pair_tok[i]), b, float(pair_p[i])))
    return slot_tok, slot_p, overflow


def _combine(ys, slot_tok, X, sel, prob, weights, bias, overflow):
    NT = X.shape[0]
    out = np.zeros((NT, OUT), dtype=np.float32)
    for c in range(NCORES):
        tok = slot_tok[c]
        valid = tok >= 0
        np.add.at(out, tok[valid], ys[c][valid].astype(np.float32))
    # bias term for every pair (device computes x @ W only)
    for k in range(KSEL):
        out += prob[:, k, None] * bias[sel[:, k]]
    # exact host fallback for capacity-overflow pairs (expected: none)
    for t, b, p in overflow:
        out[t] += p * (X[t] @ weights[b])
    return out


def _run_device(in_maps, trace=False, **kwargs):
    from concourse.bass_utils import run_bass_kernel_spmd
    return run_bass_kernel_spmd(_get_nc(), in_maps,
                                core_ids=list(range(NCORES)),
                                trace=trace, **kwargs)


def kernel(_trace=False, _bass_results=None, **inputs):
    tensor = np.asarray(inputs["tensor"], dtype=np.float32)
    sel = np.asarray(inputs["bank_selections"]).astype(np.int64)
    prob = np.asarray(inputs["bank_probabilities"], dtype=np.float32)
    weights = np.asarray(inputs["weights"], dtype=np.float32)
    bias = np.asarray(inputs["bias"], dtype=np.float32)

    NT = tensor.shape[0] * tensor.shape[1]
    X = tensor.reshape(NT, IN)
    sel2 = sel.reshape(NT, KSEL)
    prob2 = prob.reshape(NT, KSEL)

    slot_tok, slot_p, overflow = _route(X, sel2, prob2)

    in_maps = []
    for c in range(NCORES):
        tok = slot_tok[c]
        rows = X[np.where(tok >= 0, tok, 0)] * slot_p[c][:, None]
        xt = np.ascontiguousarray(rows.T)              # [IN, SLOTS] fp32
        w32 = weights[c * BPC:(c + 1) * BPC]           # (8, 512, 512) fp32
        in_maps.append({
            "xt": _bf16(_swizzle_x(xt)),
            "w": _bf16(_swizzle_w(w32)),
        })

    res = _run_device(in_maps, trace=_trace)
    if _bass_results is not None:
        _bass_results.append(res)
    ys = [res.results[c]["y"] for c in range(NCORES)]

    out = _combine(ys, slot_tok, X, sel2, prob2, weights, bias, overflow)
    return out.reshape(tensor.shape[0], tensor.shape[1], OUT)


# revision 10
# speedup vs baseline: 1.4517x; 1.3434x over previous
"""BankedLinear (MoE-style banked linear) Trainium2 kernel.

Reference computation (per token t, with k=2 selected banks):
    out[t] = sum_k prob[t,k] * (x[t] @ W[sel[t,k]] + bias[sel[t,k]])

Strategy (expert-parallel over 8 NeuronCores):
  - Core c owns banks [8c, 8c+8).  Its weight slab dominates HBM traffic;
    each bank is read exactly once system-wide (4 MB/core as bf16).
  - Host routes token-bank pairs to cores by selected bank, pre-scales each
    gathered token row by its probability, transposes to [in_feature, slot],
    and pads to CAP=32 slots per bank.
  - Precision: the harness gate is rel_err < 2e-2, so x and W are cast to
    plain bf16 (measured end-to-end error ~3e-3).  This halves the weight
    stream vs an fp32-accurate hi/lo split and needs one matmul per
    (bank, k-chunk) instead of three.
  - Weights are host-swizzled to a single [128, BPC*KC*OUT] slab whose free
    index is (bank, kc, out): any span of banks is one contiguous 2D DMA.
  - DMA plan: the 8 per-bank weight DMAs (512 KB each) stream FIFO on the
    sync (SP HWDGE) ring at ~350 GB/s; the x dispatch and the per-quad
    output stores ride the scalar (ACT HWDGE) ring so they never stall the
    weight stream.
  - PSUM: four banks share one [128, OUT] PSUM tile at partition offsets
    0/32/64/96 (tile_position column groups), so one full-width DVE cast
    and one 128 KB store cover four banks.
  - PE warm-up: the PE clock sits at 1.2 GHz until the HAM sees ~3.4 us of
    sustained activity.  A burst of dummy matmuls on a zeroed scratch tile
    warms it during the DMA head so real matmuls run at 2.4 GHz and keep
    pace with the weight stream.
  - Bias is folded in on the host (one gather + multiply-add over 1024
    pairs); host scatter-adds the per-pair device results into the output.

Fixed shapes: B=2, T=256, K=2, IN=OUT=512, NB=64 banks, 8 cores.
Capacity: 32 slots/bank (binomial mean 16, sd ~4; overflow pairs — none for
realistic routing — are handled exactly on the host as a fallback).
"""

import numpy as np
from contextlib import ExitStack

B, T, KSEL = 2, 256, 2
IN, OUT, NB = 512, 512, 64
NCORES = 8
BPC = NB // NCORES          # banks per core = 8
CAP = 32                    # padded token slots per bank
SLOTS = BPC * CAP           # 256 dispatch rows per core
PCHUNK = 128                # contraction chunk (SBUF partition dim)
KC = IN // PCHUNK           # 4 contraction chunks
QUADS = BPC // 4            # 4 banks share one PSUM tile / output store
NWARM = 10                  # dummy matmuls to lift the HAM clock gate

_cache = {}


def _build_nc():
    """Build the Bass/Tile program (one SPMD NeuronCore program)."""
    import concourse.tile as tile
    import concourse.mybir as mybir
    from concourse import bacc

    f32 = mybir.dt.float32
    bf16 = mybir.dt.bfloat16
    nc = bacc.Bacc("TRN2", target_bir_lowering=False, debug=False,
                   num_devices=NCORES)
    # host-pre-swizzled SBUF layouts: partition dim first, contiguous free dim
    xt = nc.dram_tensor("xt", [PCHUNK, KC * SLOTS], bf16,
                        kind="ExternalInput").ap()
    w = nc.dram_tensor("w", [PCHUNK, BPC * KC * OUT], bf16,
                       kind="ExternalInput").ap()
    y = nc.dram_tensor("y", [SLOTS, OUT], bf16, kind="ExternalOutput").ap()

    from concourse.tile import add_dep_helper

    def chain(dep_chain, binst, reason):
        # pin scheduler order: binst depends on the previous link
        if dep_chain:
            add_dep_helper(binst.ins, dep_chain[-1].ins, sync=False,
                           reason=reason)
        dep_chain.append(binst)

    with tile.TileContext(nc) as tc:
        with ExitStack() as ctx:
            xpool = ctx.enter_context(tc.tile_pool(name="xp", bufs=1))
            wpool = ctx.enter_context(tc.tile_pool(name="wp", bufs=BPC))
            ypool = ctx.enter_context(tc.tile_pool(name="yp", bufs=BPC + 2))
            pspool = ctx.enter_context(
                tc.tile_pool(name="ps", bufs=1, space="PSUM"))

            # x dispatch rides the scalar (ACT) ring, concurrent with the
            # weight stream on the sync (SP) ring
            x_sb = xpool.tile([PCHUNK, KC * SLOTS], bf16, tag="x")
            sq = []    # scalar-ring chain: x, then per-quad y stores
            chain(sq, nc.scalar.dma_start(x_sb[:], xt[:]), "x first on ACT")

            # Weight stream: one 512 KB DMA per bank, except the LAST bank
            # which arrives as four 128 KB kc-chunks so its matmuls can
            # chase the stream and the tail only waits on the final chunk.
            wq = []    # sync-ring DMA chain (keeps FIFO = compute order)
            wts = []   # per bank: list of (tile, col_offset) per kc chunk
            for j in range(BPC):
                if j < BPC - 1:
                    w_t = wpool.tile([PCHUNK, KC * OUT], bf16, tag="w",
                                     name=f"w{j}")
                    chain(wq, nc.sync.dma_start(
                        w_t[:], w[:, j * KC * OUT:(j + 1) * KC * OUT]),
                        "weight ring order")
                    wts.append([(w_t, kc * OUT) for kc in range(KC)])
                else:
                    chunks = []
                    for kc in range(KC):
                        w_t = wpool.tile([PCHUNK, OUT], bf16, tag="wk",
                                         name=f"w{j}k{kc}")
                        c0 = (j * KC + kc) * OUT
                        chain(wq, nc.sync.dma_start(
                            w_t[:], w[:, c0:c0 + OUT]), "weight ring order")
                        chunks.append((w_t, 0))
                    wts.append(chunks)

            # output staging: banks 0-3 and 4-6 in two group tiles (their
            # casts/stores hide mid-stream); bank 7 in two column halves
            # so only its short chain sits on the kernel tail
            ysbA = ypool.tile([128, OUT], bf16, tag="y", name="ysbA")
            ysbB = ypool.tile([96, OUT], bf16, tag="y", name="ysbB")
            y7a = ypool.tile([CAP, OUT // 2], bf16, tag="y7", name="y7a")
            y7b = ypool.tile([CAP, OUT // 2], bf16, tag="y7", name="y7b")

            # PE warm-up: dummy matmuls on a zeroed scratch tile, ordered
            # before the real matmuls.  They run during the DMA head and
            # flip the HAM clock gate from 1.2 to 2.4 GHz.
            warm_x = xpool.tile([PCHUNK, OUT], bf16, tag="wx")
            warm_ps = pspool.tile([CAP, OUT], f32, tag="wps")
            nc.gpsimd.memset(warm_x[:], 0.0)
            mq = []    # PE order chain: warm-up first, then bank order
            for i in range(NWARM):
                mm = nc.tensor.matmul(warm_ps[:], warm_x[:, :CAP],
                                      warm_x[:, :OUT],
                                      start=True, stop=True,
                                      skip_group_check=True)
                chain(mq, mm, "warmup order")

            # Four banks per quad: bank b lands in PE column group b
            # (tile_position) and PSUM partitions 32b..32b+32 of ONE shared
            # [128, OUT] PSUM tile, so a single full-width DVE cast and one
            # 128 KB store cover the whole quad.
            # Bank groups: {0-3} and {4-6} share one PSUM tile each (bank b
            # in PE column group b, PSUM partitions 32b..32b+32); bank 7
            # gets its OWN tile so its tail never WARs a shared tile.
            def bank_matmuls(j, ps, b):
                for kc in range(KC):
                    xs = slice(kc * SLOTS + j * CAP,
                               kc * SLOTS + (j + 1) * CAP)
                    w_t, c0 = wts[j][kc]
                    mm = nc.tensor.matmul(
                        ps[b * CAP:(b + 1) * CAP, :],
                        x_sb[:, xs], w_t[:, c0:c0 + OUT],
                        start=(kc == 0), stop=(kc == KC - 1),
                        tile_position=(0, b * CAP),
                        skip_group_check=True)
                    if kc == 0:
                        chain(mq, mm, "bank compute order")

            ps0 = pspool.tile([128, OUT], f32, tag="ps0", name="ps0")
            for b in range(4):
                bank_matmuls(b, ps0, b)
            nc.vector.tensor_copy(ysbA[:], ps0[:])
            chain(sq, nc.scalar.dma_start(y[0:128, :], ysbA[:]),
                  "y store order")

            ps1 = pspool.tile([96, OUT], f32, tag="ps1", name="ps1")
            for b in range(3):
                bank_matmuls(4 + b, ps1, b)
            nc.vector.tensor_copy(ysbB[:], ps1[:])
            chain(sq, nc.scalar.dma_start(y[128:224, :], ysbB[:]),
                  "y store order")

            ps7 = pspool.tile([CAP, OUT], f32, tag="ps7", name="ps7")
            bank_matmuls(BPC - 1, ps7, 0)
            # column-split tail: store of the first half overlaps the
            # cast of the second half
            h = OUT // 2
            nc.vector.tensor_copy(y7a[:], ps7[:, 0:h])
            chain(sq, nc.scalar.dma_start(
                y[SLOTS - CAP:SLOTS, 0:h], y7a[:]), "y store order")
            nc.vector.tensor_copy(y7b[:], ps7[:, h:OUT])
            chain(sq, nc.scalar.dma_start(
                y[SLOTS - CAP:SLOTS, h:OUT], y7b[:]), "y store order")
    nc.compile()
    return nc


def _get_nc():
    if "nc" not in _cache:
        _cache["nc"] = _build_nc()
    return _cache["nc"]


def _bf16(a32):
    import ml_dtypes
    return a32.astype(ml_dtypes.bfloat16)


def _swizzle_x(xt):
    """[IN, SLOTS] -> [128, KC*SLOTS] with free index (kc, slot)."""
    return np.ascontiguousarray(
        xt.reshape(KC, PCHUNK, SLOTS).transpose(1, 0, 2).reshape(
            PCHUNK, KC * SLOTS))


def _swizzle_w(w):
    """[BPC, IN, OUT] -> [128, BPC*KC*OUT] with free index (bank, kc, out)."""
    return np.ascontiguousarray(
        w.reshape(BPC, KC, PCHUNK, OUT).transpose(2, 0, 1, 3).reshape(
            PCHUNK, BPC * KC * OUT))


def _route(X, sel, prob):
    """Group token-bank pairs by bank, build per-core dispatch arrays.

    Returns (slot_tok [NCORES,SLOTS] int64 (-1=pad), slot_p, overflow list
    of (token, bank, prob))."""
    NT = X.shape[0]
    pair_tok = np.repeat(np.arange(NT, dtype=np.int64), KSEL)
    pair_bank = sel.reshape(-1)
    pair_p = prob.reshape(-1)

    order = np.argsort(pair_bank, kind="stable")
    counts = np.bincount(pair_bank, minlength=NB)
    starts = np.concatenate(([0], np.cumsum(counts)))

    slot_tok = np.full((NCORES, SLOTS), -1, dtype=np.int64)
    slot_p = np.zeros((NCORES, SLOTS), dtype=np.float32)
    overflow = []
    for b in range(NB):
        c, j = divmod(b, BPC)
        s0, s1 = starts[b], starts[b + 1]
        take = min(s1 - s0, CAP)
        idx = order[s0:s0 + take]
        slot_tok[c, j * CAP: j * CAP + take] = pair_tok[idx]
        slot_p[c, j * CAP: j * CAP + take] = pair_p[idx]
        for i in order[s0 + take:s1]:
            overflow.append((int(pair_tok[i]), b, float(pair_p[i])))
    return slot_tok, slot_p, overflow


def _combine(ys, slot_tok, X, sel, prob, weights, bias, overflow):
    NT = X.shape[0]
    out = np.zeros((NT, OUT), dtype=np.float32)
    for c in range(NCORES):
        tok = slot_tok[c]
        valid = tok >= 0
        np.add.at(out, tok[valid], ys[c][valid].astype(np.float32))
    # bias term for every pair (device computes x @ W only)
    for k in range(KSEL):
        out += prob[:, k, None] * bias[sel[:, k]]
    # exact host fallback for capacity-overflow pairs (expected: none)
    for t, b, p in overflow:
        out[t] += p * (X[t] @ weights[b])
    return out


def _run_device(in_maps, trace=False, **kwargs):
    from concourse.bass_utils import run_bass_kernel_spmd
    return run_bass_kernel_spmd(_get_nc(), in_maps,
                                core_ids=list(range(NCORES)),
                                trace=trace, **kwargs)


def kernel(_trace=False, _bass_results=None, **inputs):
    tensor = np.asarray(inputs["tensor"], dtype=np.float32)
    sel = np.asarray(inputs["bank_selections"]).astype(np.int64)
    prob = np.asarray(inputs["bank_probabilities"], dtype=np.float32)
    weights = np.asarray(inputs["weights"], dtype=np.float32)
    bias = np.asarray(inputs["bias"], dtype=np.float32)

    NT = tensor.shape[0] * tensor.shape[1]
    X = tensor.reshape(NT, IN)
    sel2 = sel.reshape(NT, KSEL)
    prob2 = prob.reshape(NT, KSEL)

    slot_tok, slot_p, overflow = _route(X, sel2, prob2)

    in_maps = []
    for c in range(NCORES):
        tok = slot_tok[c]
        rows = X[np.where(tok >= 0, tok, 0)] * slot_p[c][:, None]
        xt = np.ascontiguousarray(rows.T)              # [IN, SLOTS] fp32
        w32 = weights[c * BPC:(c + 1) * BPC]           # (8, 512, 512) fp32
        in_maps.append({
            "xt": _bf16(_swizzle_x(xt)),
            "w": _bf16(_swizzle_w(w32)),
        })

    res = _run_device(in_maps, trace=_trace)
    if _bass_results is not None:
        _bass_results.append(res)
    ys = [res.results[c]["y"] for c in range(NCORES)]

    out = _combine(ys, slot_tok, X, sel2, prob2, weights, bias, overflow)
    return out.reshape(tensor.shape[0], tensor.shape[1], OUT)
